# revision 68
# baseline (speedup 1.0000x reference)
"""Trainium2 Bass kernel for backward-chaining grounder (nn_BCGrounder).

Sharding: batch-parallel over 8 cores (one batch of 64 states per core).
The fact/rule KB is replicated. Everything is computed on-device; the host
only shards inputs, pads the fact table, and stacks per-core outputs.
"""
import sys
sys.path.insert(0, "/opt/trn_rl_repo")

import numpy as np

import concourse.bass as bass
import concourse.mybir as mybir
from concourse import bacc
from concourse.tile import TileContext
import concourse.bass_interp as _bi
_bi.get_valid_libraries_for_instruction = lambda inst: None

F32 = mybir.dt.float32
I32 = mybir.dt.int32
I16 = mybir.dt.int16
U8 = mybir.dt.uint8
U32 = mybir.dt.uint32
BF16 = mybir.dt.bfloat16
OP = mybir.AluOpType
AF = mybir.ActivationFunctionType

# problem constants
B, S, G = 8, 64, 8
F, R, MB = 100000, 64, 3
K_F, K_R = 64, 16
PAD = 6000
CONST_NO = 5000
PADFACT = 65535  # pad fact component: digits (15,15,15,15), unmatchable in-spec

BLK = 512           # facts per matmul psum half-tile
IBLK = 1024         # facts per indicator/IND block
NSLOT = 64          # output slots per batch


def chunk_sizes(Fp, nchunk):
    # ramped chunks to shorten pipeline fill; sizes in IBLK units
    total = Fp // IBLK
    if False:
        pass
    else:
        base = total // nchunk
        units = [base] * (nchunk - 1) + [total - base * (nchunk - 1)]
    assert sum(units) == total and all(u > 0 for u in units)
    return [u * IBLK for u in units]


def build_kernel(Fp=100352, nchunk=4):
    """Build the SPMD single-core program. Fp must be divisible by 512*nchunk
    and by 128*nchunk*... (chunk raw layout)."""
    nblk = Fp // BLK              # 512-granular count blocks
    nblk2 = Fp // IBLK            # 1024-wide compute tiles
    cchunks = chunk_sizes(Fp, nchunk)   # list of chunk sizes (facts)

    nc = bacc.Bacc()

    # ---------------- dram parameters ----------------
    pg_in = nc.declare_dram_parameter("pg", [S, G * 3], I32, isOutput=False)
    sc_in = nc.declare_dram_parameter("scores", [1, S], F32, isOutput=False)
    facts_in = nc.declare_dram_parameter("facts", [Fp, 3], I32, isOutput=False)
    heads_in = nc.declare_dram_parameter("heads", [R, 3], I32, isOutput=False)
    bodies_in = nc.declare_dram_parameter("bodies", [R, 9], I32, isOutput=False)
    lens_in = nc.declare_dram_parameter("lens", [R, 1], I32, isOutput=False)

    goals_out = nc.declare_dram_parameter("goals", [NSLOT, G * 3], I32, isOutput=True)
    gbody_out = nc.declare_dram_parameter("gbody", [NSLOT, MB * 3], I32, isOutput=True)
    succ_out = nc.declare_dram_parameter("succ", [1, NSLOT], I32, isOutput=True)
    scout_out = nc.declare_dram_parameter("scout", [1, NSLOT], F32, isOutput=True)


    # dram intermediates
    ind_dram = nc.dram_tensor("ind_dram", [S * nblk, BLK], U8)
    strec_dram = nc.dram_tensor("strec_dram", [S, 64], F32)
    CW = ((nblk + 63) // 64) * 64
    cum_dram = nc.dram_tensor("cum_dram", [S, CW], F32)
    ridx_dram = nc.dram_tensor("ridx_dram", [S, 64], F32)
    rrec_dram = nc.dram_tensor("rrec_dram", [R, 64], I32)

    factsg_in = nc.declare_dram_parameter("factsg", [Fp // 16, 64], I32, isOutput=False)
    sidx_dram = nc.dram_tensor("sidx_dram", [1, NSLOT], I16)
    grow_dram = nc.dram_tensor("grow_dram", [1, NSLOT], I16)
    frow_dram = nc.dram_tensor("frow_dram", [1, NSLOT], I16)
    rrow_dram = nc.dram_tensor("rrow_dram", [1, NSLOT], I16)

    with TileContext(nc) as tc:
        import contextlib
        ctx = contextlib.ExitStack()
        with ctx:
            small = ctx.enter_context(tc.tile_pool(name="small", bufs=1))
            feat_pool = ctx.enter_context(tc.tile_pool(name="feat", bufs=2))
            raw_pool = ctx.enter_context(tc.tile_pool(name="raw", bufs=2))
            nt_pool = ctx.enter_context(tc.tile_pool(name="nt", bufs=4))
            stg_pool = ctx.enter_context(tc.tile_pool(name="stg", bufs=3))
            psum = ctx.enter_context(tc.tile_pool(name="ps", bufs=2, space="PSUM"))
            ntps_pool = ctx.enter_context(tc.tile_pool(name="ntps", bufs=2, space="PSUM"))
            psum2 = ctx.enter_context(tc.tile_pool(name="ps2", bufs=1, space="PSUM"))

            # shared index rows via DVE scan (avoid gpsimd iota)
            ones512 = small.tile([S, IBLK], F32)
            nc.vector.memset(ones512, 1.0)
            zer512 = small.tile([S, IBLK], F32)
            nc.vector.memset(zer512, 0.0)
            iota512 = small.tile([S, IBLK], F32)
            nc.vector.tensor_tensor_scan(iota512, ones512, zer512, -1.0,
                                         op0=OP.add, op1=OP.add)

            # =========== per-state query prep ===========
            pg = small.tile([S, G * 3], I32)
            nc.sync.dma_start(pg, pg_in[:, :])
            pgf = small.tile([S, G * 3], F32)
            nc.vector.tensor_copy(pgf, pg)
            # remaining = pg with atom0 -> PAD
            rem = small.tile([S, G * 3], F32)
            nc.vector.tensor_copy(rem, pgf)
            nc.vector.memset(rem[:, 0:3], float(PAD))

            qp = pgf[:, 0:1]
            qa0 = pgf[:, 1:2]
            qa1 = pgf[:, 2:3]

            qv = small.tile([S, 8], F32)  # va0, va1, active, notv0, notv1, qnp0, qnp1, one
            va0, va1 = qv[:, 0:1], qv[:, 1:2]
            act01 = qv[:, 2:3]
            nv0, nv1 = qv[:, 3:4], qv[:, 4:5]
            qnp0, qnp1 = qv[:, 5:6], qv[:, 6:7]

            t1 = small.tile([S, 4], F32)
            # va = (qa > 5000) & (qa != 6000)
            nc.vector.tensor_scalar(t1[:, 0:1], qa0, float(CONST_NO), None, op0=OP.is_gt)
            nc.vector.tensor_scalar(t1[:, 1:2], qa0, float(PAD), None, op0=OP.not_equal)
            nc.vector.tensor_tensor(va0, t1[:, 0:1], t1[:, 1:2], op=OP.mult)
            nc.vector.tensor_scalar(t1[:, 2:3], qa1, float(CONST_NO), None, op0=OP.is_gt)
            nc.vector.tensor_scalar(t1[:, 3:4], qa1, float(PAD), None, op0=OP.not_equal)
            nc.vector.tensor_tensor(va1, t1[:, 2:3], t1[:, 3:4], op=OP.mult)
            nc.vector.tensor_scalar(act01, qp, float(PAD), None, op0=OP.not_equal)
            nc.vector.tensor_scalar(nv0, va0, -1.0, 1.0, op0=OP.mult, op1=OP.add)
            nc.vector.tensor_scalar(nv1, va1, -1.0, 1.0, op0=OP.mult, op1=OP.add)
            nc.vector.tensor_scalar(qnp0, qa0, float(PAD), None, op0=OP.not_equal)
            nc.vector.tensor_scalar(qnp1, qa1, float(PAD), None, op0=OP.not_equal)

            # --- digits of (qp, qa0, qa1): qi [S,3] int; qd [S,12] f32 ---
            qi = small.tile([S, 3], I32)
            nc.vector.tensor_copy(qi, pg[:, 0:3])
            qsh = small.tile([S, 12], I32)
            for c in range(3):
                x = qi[:, c:c + 1]
                nc.vector.tensor_scalar(qsh[:, 4 * c + 0:4 * c + 1], x, 0, 15,
                                        op0=OP.arith_shift_right, op1=OP.bitwise_and)
                nc.vector.tensor_scalar(qsh[:, 4 * c + 1:4 * c + 2], x, 4, 15,
                                        op0=OP.arith_shift_right, op1=OP.bitwise_and)
                nc.vector.tensor_scalar(qsh[:, 4 * c + 2:4 * c + 3], x, 8, 15,
                                        op0=OP.arith_shift_right, op1=OP.bitwise_and)
                nc.vector.tensor_scalar(qsh[:, 4 * c + 3:4 * c + 4], x, 12, 15,
                                        op0=OP.arith_shift_right, op1=OP.bitwise_and)
            qd = small.tile([S, 12], F32)
            nc.vector.tensor_copy(qd, qsh)

            # w per digit-column: wrep [S,12] = [1,1,1,1, nv0*4, nv1*4]
            wrep = small.tile([S, 12], F32)
            nc.vector.memset(wrep[:, 0:4], 1.0)
            nc.vector.tensor_copy(wrep[:, 4:8], nv0.to_broadcast([S, 4]))
            nc.vector.tensor_copy(wrep[:, 8:12], nv1.to_broadcast([S, 4]))

            # M_T [S,24]: even col 2k = -2*w*qd_k ; odd col = w
            mt = small.tile([S, 24], F32)
            mwork = small.tile([S, 12], F32)
            nc.vector.tensor_tensor(mwork, wrep, qd, op=OP.mult)
            nc.vector.tensor_scalar(mt[:, 0:24:2], mwork, -2.0, None, op0=OP.mult)
            nc.vector.tensor_copy(mt[:, 1:24:2], wrep)
            # Qw = sum w*qd^2 ; bias = act ? 1+Qw : -1e6
            qd2 = small.tile([S, 12], F32)
            nc.vector.tensor_tensor(qd2, mwork, qd, op=OP.mult)
            qw = small.tile([S, 1], F32)
            nc.vector.tensor_reduce(qw, qd2, op=OP.add, axis=mybir.AxisListType.X)
            biasb = small.tile([S, 1], F32)
            nc.vector.tensor_scalar(biasb, qw, -1.0, 1.0, op0=OP.mult, op1=OP.add)
            nc.vector.tensor_tensor(biasb, biasb, act01, op=OP.mult)
            tneg = small.tile([S, 1], F32)
            nc.vector.tensor_scalar(tneg, act01, -1.0, 1.0, op0=OP.mult, op1=OP.add)
            nc.vector.tensor_scalar(tneg, tneg, -1.0e6, None, op0=OP.mult)
            nc.vector.tensor_tensor(biasb, biasb, tneg, op=OP.add)
            # DVE indicator scalar: active ? -Qw : 1e9 (sentinel, never equal)
            negqw = small.tile([S, 1], F32)
            nc.vector.tensor_scalar(negqw, qw, -1.0, None, op0=OP.mult)
            nc.vector.tensor_tensor(negqw, negqw, act01, op=OP.mult)
            tpos = small.tile([S, 1], F32)
            nc.vector.tensor_scalar(tpos, act01, -1.0, 1.0, op0=OP.mult, op1=OP.add)
            nc.vector.tensor_scalar(tpos, tpos, 1.0e9, None, op0=OP.mult)
            nc.vector.tensor_tensor(negqw, negqw, tpos, op=OP.add)

            # M lhsT [24, S] bf16 via PE transpose
            from concourse.masks import make_identity
            ident = small.tile([S, S], F32)
            make_identity(nc, ident)
            ident1 = small.tile([1, 1], F32)
            nc.vector.memset(ident1, 1.0)

            def pe_transpose(dst, src, identity, label, via=None):
                pps = psum2.tile([dst.shape[0], dst.shape[1]],
                                 src.dtype, tag="miscps")
                nc.tensor.transpose(pps, src, identity)
                (via or nc.vector).tensor_copy(dst, pps)

            mt_ps = psum2.tile([24, S], F32, tag="miscps")
            nc.tensor.transpose(mt_ps, mt, ident)
            mmat = small.tile([24, S], BF16)
            nc.scalar.activation(mmat, mt_ps, AF.Copy)
            mm2 = small.tile([56, S], BF16)
            nc.sync.dma_start(mm2[0:24, :], mmat)
            nc.sync.dma_start(mm2[32:56, :], mmat)

            # =========== rules ===========
            hd = small.tile([R, 3], I32)
            nc.sync.dma_start(hd, heads_in[:, :])
            hdf = small.tile([R, 3], F32)
            nc.vector.tensor_copy(hdf, hd)
            # head var flags per rule [R,2]
            hvf = small.tile([R, 2], F32)
            ht = small.tile([R, 2], F32)
            for a in range(2):
                nc.vector.tensor_scalar(ht[:, a:a + 1], hdf[:, 1 + a:2 + a],
                                        float(CONST_NO), None, op0=OP.is_gt)
                nc.vector.tensor_scalar(hvf[:, a:a + 1], hdf[:, 1 + a:2 + a],
                                        float(PAD), None, op0=OP.not_equal)
            nc.vector.tensor_tensor(hvf, hvf, ht, op=OP.mult)

            # transpose head fields + var flags into rows [5, R] then bcast to [S, R]
            hd5 = small.tile([R, 5], F32)
            nc.vector.tensor_copy(hd5[:, 0:3], hdf)
            nc.vector.tensor_copy(hd5[:, 3:5], hvf)
            onesrow = small.tile([1, NSLOT], F32)
            nc.vector.memset(onesrow, 1.0)
            hrow5 = small.tile([1, 5 * R], F32)
            for rr in range(5):
                pe_transpose(hrow5[:, rr * R:(rr + 1) * R], hd5[:, rr:rr + 1],
                             ident, "hd5_%d" % rr)
            hb_ps = psum2.tile([S, 5 * R], F32, tag="miscps")
            nc.tensor.matmul(hb_ps, onesrow[:, 0:S], hrow5, start=True, stop=True)
            hb = small.tile([S, 5 * R], F32)
            nc.scalar.activation(hb, hb_ps, AF.Copy)
            hpb = hb[:, 0 * R:1 * R]
            h1b = hb[:, 1 * R:2 * R]
            h2b = hb[:, 2 * R:3 * R]
            hv1b = hb[:, 3 * R:4 * R]
            hv2b = hb[:, 4 * R:5 * R]

            rm = small.tile([S, R], F32)
            rt = small.tile([S, R], F32)
            nc.vector.tensor_scalar(rm, hpb, qp, None, op0=OP.is_equal)
            nc.vector.tensor_tensor(rm, rm, act01.to_broadcast([S, R]), op=OP.mult)
            # arg0 clause: hv1 | va0 | eq
            nc.vector.tensor_scalar(rt, h1b, qa0, None, op0=OP.is_equal)
            nc.vector.tensor_tensor(rt, rt, hv1b, op=OP.max)
            nc.vector.tensor_tensor(rt, rt, va0.to_broadcast([S, R]), op=OP.max)
            nc.vector.tensor_tensor(rm, rm, rt, op=OP.mult)
            # arg1 clause
            nc.vector.tensor_scalar(rt, h2b, qa1, None, op0=OP.is_equal)
            nc.vector.tensor_tensor(rt, rt, hv2b, op=OP.max)
            nc.vector.tensor_tensor(rt, rt, va1.to_broadcast([S, R]), op=OP.max)
            nc.vector.tensor_tensor(rm, rm, rt, op=OP.mult)

            rcnt = small.tile([S, 1], F32)
            rdump = small.tile([S, R], F32)
            nc.scalar.activation(rdump, rm, AF.Copy, accum_out=rcnt)

            # first-16 matching rules per state
            wrf = small.tile([S, R], F32)
            nc.vector.tensor_scalar(wrf, iota512[:, 0:R], -1.0, float(R),
                                    op0=OP.mult, op1=OP.add)
            rval = small.tile([S, R], F32)
            nc.vector.tensor_tensor(rval, rm, wrf, op=OP.mult)
            ridxf = small.tile([S, 64], F32)
            nc.vector.memset(ridxf, 0.0)
            rm8 = small.tile([S, 8], F32)
            ri8 = small.tile([S, 8], U32)
            for rnd in range(2):
                nc.vector.max(rm8, rval)
                nc.vector.max_index(ri8, rm8, rval)
                nc.vector.tensor_copy(ridxf[:, 8 * rnd:8 * rnd + 8], ri8)
                if rnd == 0:
                    nc.vector.match_replace(rval, rm8, rval, 0.0)
            nc.sync.dma_start(ridx_dram[:, :], ridxf)

            # rule record table [R,16] i32: hp,h1,h2, body9, len
            rrec = small.tile([R, 64], I32)
            nc.vector.memset(rrec, 0)
            nc.vector.tensor_copy(rrec[:, 0:3], hd)
            bod = small.tile([R, 9], I32)
            nc.sync.dma_start(bod, bodies_in[:, :])
            nc.vector.tensor_copy(rrec[:, 3:12], bod)
            lns = small.tile([R, 1], I32)
            nc.sync.dma_start(lns, lens_in[:, :])
            nc.vector.tensor_copy(rrec[:, 12:13], lns)
            nc.sync.dma_start(rrec_dram[:, :], rrec)

            # =========== facts pipeline ===========
            identb = small.tile([128, 128], BF16)
            make_identity(nc, identb)
            bcnt = small.tile([S, nblk], F32)
            fpp_max = max(cchunks) // 128
            coff = 0
            for cidx, chunk in enumerate(cchunks):
                cblk = chunk // IBLK
                fpp = chunk // 128
                raw_full = raw_pool.tile([128, fpp_max * 3], I32, tag="raw")
                raw = raw_full[:, 0:fpp * 3]
                nc.sync.dma_start(
                    raw, facts_in[coff:coff + chunk, :]
                    .rearrange("(p n) c -> p (n c)", p=128))
                feat_full = feat_pool.tile([128, fpp_max, 24], BF16, tag="feat")
                feat = feat_full[:, 0:fpp]
                dig_full = feat_pool.tile([128, 4, fpp_max], I32, tag="dig")
                dig = dig_full[:, :, 0:fpp]
                for c in range(3):
                    x = raw.rearrange("p (n c) -> p c n", c=3)[:, c]
                    for d in range(4):
                        nc.vector.tensor_scalar(dig[:, d], x, 4 * d, 15,
                                                op0=OP.arith_shift_right,
                                                op1=OP.bitwise_and)
                        fd = feat[:, :, 8 * c + 2 * d]
                        fq = feat[:, :, 8 * c + 2 * d + 1]
                        nc.gpsimd.tensor_copy(fd, dig[:, d])
                        if cidx == 0:
                            nc.scalar.activation(fq, dig[:, d], AF.Square)
                        else:
                            nc.vector.tensor_tensor(fq, dig[:, d], dig[:, d],
                                                    op=OP.mult)
                # per 1024-block: 8 transposes, split N-copy, 2 matmuls,
                # FD-split indicator (ACT cols 0:FA, DVE cols FA:IBLK)
                for tlocal in range(0, cblk, 4):
                    stg = stg_pool.tile([S, 4 * IBLK], U8)
                    nrun = min(4, cblk - tlocal)
                    for u in range(nrun):
                        t = tlocal + u
                        gblk = coff // IBLK + t
                        ntp = ntps_pool.tile([24, IBLK], BF16)
                        for c8 in range(8):
                            nc.tensor.transpose(ntp[:, c8 * 128:(c8 + 1) * 128],
                                                feat[:, 8 * t + c8, :], identb)
                        ntile = nt_pool.tile([24, IBLK], BF16)
                        nc.scalar.activation(ntile[:, 0:640], ntp[:, 0:640], AF.Copy)
                        nc.vector.tensor_copy(ntile[:, 640:IBLK], ntp[:, 640:IBLK])
                        ps = psum.tile([S, IBLK], F32)
                        nc.tensor.matmul(ps[:, 0:BLK], mmat, ntile[:, 0:BLK],
                                         start=True, stop=True)
                        nc.tensor.matmul(ps[:, BLK:IBLK], mmat, ntile[:, BLK:IBLK],
                                         start=True, stop=True)
                        so = u * IBLK
                        g2 = 2 * gblk
                        nc.scalar.activation(stg[:, so:so + BLK], ps[:, 0:BLK],
                                             AF.Relu, bias=biasb, scale=-1.0)
                        nc.vector.tensor_scalar(stg[:, so + BLK:so + IBLK],
                                                ps[:, BLK:IBLK], negqw, None,
                                                op0=OP.is_equal, op1=OP.add,
                                                accum_out=bcnt[:, g2 + 1:g2 + 2])
                        cdmp = stg_pool.tile([S, BLK], U8, tag="cdmp")
                        nc.vector.tensor_scalar(cdmp, stg[:, so:so + BLK], 0.0, None,
                                                op0=OP.add, op1=OP.add,
                                                accum_out=bcnt[:, g2:g2 + 1])
                    # spill to ind_dram rows s*nblk + (cidx*cblk + tlocal + u)
                    dst = bass.AP(ind_dram, (coff // IBLK + tlocal) * IBLK,
                                  [[nblk * BLK, S], [IBLK, nrun], [1, IBLK]])
                    eng = nc.sync if (tlocal // 4) % 2 == 0 else nc.gpsimd
                    eng.dma_start(dst, stg[:, 0:nrun * IBLK]
                                  .rearrange("s (u b) -> s u b", b=IBLK))
                coff += chunk

            # =========== pack offsets ===========
            cum = small.tile([S, nblk], F32)
            zer = small.tile([S, nblk], F32)
            nc.vector.memset(zer, 0.0)
            nc.vector.tensor_tensor_scan(cum, bcnt, zer, 0.0, op0=OP.add, op1=OP.add)
            cumpad = small.tile([S, CW], F32)
            nc.vector.memset(cumpad, 1.0e9)
            nc.vector.tensor_copy(cumpad[:, 0:nblk], cum)
            nc.sync.dma_start(cum_dram[:, :], cumpad)
            nfc = small.tile([S, 1], F32)
            nc.vector.tensor_scalar(nfc, cum[:, nblk - 1:nblk], float(K_F), None, op0=OP.min)
            nrc = small.tile([S, 1], F32)
            nc.vector.tensor_scalar(nrc, rcnt, float(K_R), None, op0=OP.min)
            tch = small.tile([S, 1], F32)
            nc.vector.tensor_tensor(tch, nfc, nrc, op=OP.add)
            # cumt over states: PE transpose [S,1]->[1,S], scan, transpose back
            trow = small.tile([1, S], F32)
            pe_transpose(trow, tch, ident, "tch")
            cumt = small.tile([1, S], F32)
            zrow = small.tile([1, S], F32)
            nc.vector.memset(zrow, 0.0)
            nc.vector.tensor_tensor_scan(cumt, trow, zrow, 0.0, op0=OP.add, op1=OP.add)
            cumtx = small.tile([1, S], F32)
            nc.vector.tensor_tensor(cumtx, cumt, trow, op=OP.subtract)
            cxcol = small.tile([S, 1], F32)
            pe_transpose(cxcol, cumtx, ident1, "cx")
            # state record
            strec = small.tile([S, 64], F32)
            nc.vector.memset(strec, 0.0)
            nc.vector.tensor_copy(strec[:, 0:3], pgf[:, 0:3])
            nc.vector.tensor_copy(strec[:, 3:4], va0)
            nc.vector.tensor_copy(strec[:, 4:5], va1)
            nc.vector.tensor_copy(strec[:, 5:6], act01)
            nc.vector.tensor_copy(strec[:, 6:7], nfc)
            nc.vector.tensor_copy(strec[:, 7:8], nrc)
            nc.vector.tensor_copy(strec[:, 8:9], tch)
            nc.vector.tensor_copy(strec[:, 9:10], cxcol)
            scrow = small.tile([1, S], F32)
            nc.sync.dma_start(scrow, sc_in[:, :])
            sccol = small.tile([S, 1], F32)
            pe_transpose(sccol, scrow, ident1, "sc")
            nc.vector.tensor_copy(strec[:, 10:11], sccol)
            nc.vector.tensor_copy(strec[:, 12:36], rem)
            nc.vector.tensor_copy(strec[:, 36:41], qv[:, 0:5])
            nc.sync.dma_start(strec_dram[:, :], strec)

            # cumt broadcast [S_slots, S_states] via PE
            ct_ps = psum2.tile([NSLOT, S], F32)
            nc.tensor.matmul(ct_ps, onesrow, cumt, start=True, stop=True)
            ctb = small.tile([NSLOT, S], F32)
            nc.scalar.activation(ctb, ct_ps, AF.Copy)

            # =========== per-slot machinery (partitions = 64 output slots) ===========
            kcol = small.tile([NSLOT, 1], F32)
            pe_transpose(kcol, iota512[0:1, 0:NSLOT], ident1, "kc")

            sk = small.tile([NSLOT, 1], F32)
            dmp = small.tile([NSLOT, S], F32)
            nc.vector.tensor_scalar(dmp, ctb, kcol, None, op0=OP.is_le, op1=OP.add, accum_out=sk)
            valid = small.tile([NSLOT, 1], F32)
            nc.vector.tensor_scalar(valid, sk, float(S), None, op0=OP.is_lt)
            nc.vector.tensor_scalar(sk, sk, float(S - 1), None, op0=OP.min)

            # gather state record by sk
            # matmul-based index wrap: wrapped[p, j] = idx[p%16 + 16j]
            # = sum_s (idx[s] * [s%16 == p%16]) * [s//16 == j]
            sdiv16 = small.tile([NSLOT, 1], F32)
            nc.vector.tensor_scalar(sdiv16, kcol, 1.0 / 16.0, None, op0=OP.mult)
            sdi = small.tile([NSLOT, 1], I32)
            nc.vector.tensor_copy(sdi, sdiv16)
            nc.vector.tensor_copy(sdiv16, sdi)
            smod = small.tile([NSLOT, 1], F32)
            nc.vector.tensor_scalar(smod, sdiv16, -16.0, None, op0=OP.mult)
            nc.vector.tensor_tensor(smod, smod, kcol, op=OP.add)
            pmod = small.tile([NSLOT, 128], F32)
            nc.vector.tensor_scalar(pmod, iota512[:, 0:128], 1.0 / 16.0, None,
                                    op0=OP.mult)
            pmi = small.tile([NSLOT, 128], I32)
            nc.vector.tensor_copy(pmi, pmod)
            nc.vector.tensor_copy(pmod, pmi)
            nc.vector.tensor_scalar(pmod, pmod, -16.0, None, op0=OP.mult)
            nc.vector.tensor_tensor(pmod, pmod, iota512[:, 0:128], op=OP.add)
            permw = small.tile([NSLOT, 128], F32)
            nc.vector.tensor_scalar(permw, pmod, smod, None, op0=OP.is_equal)
            jsel = small.tile([NSLOT, 4], F32)
            nc.vector.tensor_scalar(jsel, iota512[:, 0:4], sdiv16, None,
                                    op0=OP.is_equal)

            n64reg = nc.alloc_register(mybir.EngineType.Pool, "n64")
            nc.gpsimd.reg_mov(n64reg, NSLOT)

            dram_by_label = {"sidx": sidx_dram, "grow": grow_dram,
                             "frow": frow_dram, "rrow": rrow_dram}

            def make_idxt(col_f32, clampmax, label):
                cl = small.tile([NSLOT, 1], F32, tag="cl_" + label)
                nc.vector.tensor_scalar(cl, col_f32, float(clampmax), 0.0,
                                        op0=OP.min, op1=OP.max)
                ci16 = small.tile([NSLOT, 1], I16, tag="ci_" + label)
                nc.vector.tensor_copy(ci16, cl)
                dram16 = dram_by_label[label]
                nc.sync.dma_start(dram16[:, :], ci16)
                idxt = small.tile([128, 4], I16, tag="gidx_" + label)
                engs = [nc.sync, nc.gpsimd, nc.scalar]
                for grp in range(8):
                    engs[grp % 3].dma_start(idxt[16 * grp:16 * grp + 16, :],
                                            bass.AP(dram16, 0, [[1, 16], [16, 4]]))
                return idxt

            def gather16(idxt, src_ap, esize, dt, label):
                out = small.tile([128, 1, esize], dt, tag="gout_" + label)
                nc.gpsimd.dma_gather(out, src_ap, idxt, NSLOT, n64reg, esize)
                return out[0:NSLOT, 0, :]

            sidxt = make_idxt(sk, S - 1, "sidx")
            srg = gather16(sidxt, strec_dram[:, :], 64, F32, "strec")
            cmg_full = gather16(sidxt, cum_dram[:, :], CW, F32, "cum")
            cmg = cmg_full[:, 0:nblk]

            g_qp = srg[:, 0:1]
            g_qa0 = srg[:, 1:2]
            g_qa1 = srg[:, 2:3]
            g_va0 = srg[:, 3:4]
            g_va1 = srg[:, 4:5]
            g_nf = srg[:, 6:7]
            g_cx = srg[:, 9:10]
            g_sc = srg[:, 10:11]
            g_rem = srg[:, 12:36]

            jk = small.tile([NSLOT, 1], F32)
            nc.vector.tensor_tensor(jk, kcol, g_cx, op=OP.subtract)
            isfact = small.tile([NSLOT, 1], F32)
            nc.vector.tensor_tensor(isfact, jk, g_nf, op=OP.is_lt)
            nc.vector.tensor_tensor(isfact, isfact, valid, op=OP.mult)
            isrule = small.tile([NSLOT, 1], F32)
            nc.vector.tensor_tensor(isrule, isfact, valid, op=OP.subtract)
            nc.vector.tensor_scalar(isrule, isrule, -1.0, None, op0=OP.mult)
            jr = small.tile([NSLOT, 1], F32)
            nc.vector.tensor_tensor(jr, jk, g_nf, op=OP.subtract)

            # block of the jk-th match: nb = sum(cum <= jk)
            nb = small.tile([NSLOT, 1], F32)
            dmp2 = small.tile([NSLOT, nblk], F32)
            nc.vector.tensor_scalar(dmp2, cmg, jk, None, op0=OP.is_le, op1=OP.add, accum_out=nb)
            nc.vector.tensor_scalar(nb, nb, float(nblk - 1), None, op0=OP.min)
            # cumExcl at that block = cum[nb-1] (0 if nb==0)
            i196f = iota512[:, 0:nblk]
            nbm1 = small.tile([NSLOT, 1], F32)
            nc.vector.tensor_scalar(nbm1, nb, 1.0, None, op0=OP.subtract)
            oh = small.tile([NSLOT, nblk], F32)
            nc.vector.tensor_scalar(oh, i196f, nbm1, None, op0=OP.is_equal)
            nc.vector.tensor_tensor(oh, oh, cmg, op=OP.mult)
            cxb = small.tile([NSLOT, 1], F32)
            nc.vector.tensor_reduce(cxb, oh, op=OP.add, axis=mybir.AxisListType.X)
            rk = small.tile([NSLOT, 1], F32)
            nc.vector.tensor_tensor(rk, jk, cxb, op=OP.subtract)

            # gather IND block: row = sk*nblk + nb
            grow = small.tile([NSLOT, 1], F32)
            nc.vector.tensor_scalar(grow, sk, float(nblk), None, op0=OP.mult)
            nc.vector.tensor_tensor(grow, grow, nb, op=OP.add)
            gidxt = make_idxt(grow, S * nblk - 1, "grow")
            indg8 = gather16(gidxt, ind_dram[:, :], BLK, U8, "ind")
            indb = small.tile([NSLOT, BLK], F32)
            nc.vector.tensor_copy(indb, indg8)

            # position of the (rk+1)-th match: prefix-count then count(pc <= rk)
            pcnt = small.tile([NSLOT, BLK], F32)
            nc.vector.tensor_tensor_scan(pcnt, indb, zer512[:, 0:BLK], 0.0,
                                         op0=OP.add, op1=OP.add)
            pos = small.tile([NSLOT, 1], F32)
            pdmp = small.tile([NSLOT, BLK], F32)
            nc.vector.tensor_scalar(pdmp, pcnt, rk, None, op0=OP.is_le,
                                    op1=OP.add, accum_out=pos)
            fidx = small.tile([NSLOT, 1], F32)
            nc.vector.tensor_scalar(fidx, nb, float(BLK), None, op0=OP.mult)
            nc.vector.tensor_tensor(fidx, fidx, pos, op=OP.add)

            # gather fact triple: row = fidx>>4 (16 facts of 4 ints per row)
            frow = small.tile([NSLOT, 1], F32)
            nc.vector.tensor_scalar(frow, fidx, 1.0 / 16.0, None, op0=OP.mult)
            fri = small.tile([NSLOT, 1], I32)
            nc.vector.tensor_copy(fri, frow)  # trunc
            frf = small.tile([NSLOT, 1], F32)
            nc.vector.tensor_copy(frf, fri)
            fidxt = make_idxt(frf, Fp // 16 - 1, "frow")
            f64 = gather16(fidxt, factsg_in[:, :], 64, I32, "fact")
            f64f = small.tile([NSLOT, 64], F32)
            nc.vector.tensor_copy(f64f, f64)
            # j16 = fidx - 16*floor(fidx/16); triple at cols 4*j16+{0,1,2}
            j8 = small.tile([NSLOT, 1], F32)
            nc.vector.tensor_scalar(j8, frf, -16.0, None, op0=OP.mult)
            nc.vector.tensor_tensor(j8, j8, fidx, op=OP.add)
            j83 = small.tile([NSLOT, 1], F32)
            nc.vector.tensor_scalar(j83, j8, 4.0, None, op0=OP.mult)
            i24f = iota512[:, 0:64]
            ft = small.tile([NSLOT, 3], F32)
            ohf = small.tile([NSLOT, 64], F32)
            sel3 = small.tile([NSLOT, 1], F32)
            for cc in range(3):
                nc.vector.tensor_scalar(sel3, j83, float(cc), None, op0=OP.add)
                nc.vector.tensor_scalar(ohf, i24f, sel3, None, op0=OP.is_equal)
                nc.vector.tensor_tensor(ohf, ohf, f64f, op=OP.mult)
                nc.vector.tensor_reduce(ft[:, cc:cc + 1], ohf, op=OP.add,
                                        axis=mybir.AxisListType.X)

            # rule slot: rule id
            rig_full = gather16(sidxt, ridx_dram[:, :], 64, F32, "ridx")
            rig = rig_full[:, 0:16]
            jrc = small.tile([NSLOT, 1], F32)
            nc.vector.tensor_scalar(jrc, jr, float(K_R - 1), 0.0, op0=OP.min, op1=OP.max)
            i16f = iota512[:, 0:16]
            ohx = small.tile([NSLOT, 16], F32)
            nc.vector.tensor_scalar(ohx, i16f, jrc, None, op0=OP.is_equal)
            nc.vector.tensor_tensor(ohx, ohx, rig, op=OP.mult)
            rid = small.tile([NSLOT, 1], F32)
            nc.vector.tensor_reduce(rid, ohx, op=OP.add, axis=mybir.AxisListType.X)
            ridxt = make_idxt(rid, R - 1, "rrow")
            rrg_full = gather16(ridxt, rrec_dram[:, :], 64, I32, "rrec")
            rrg = rrg_full[:, 0:16]
            rrf = small.tile([NSLOT, 16], F32)
            nc.vector.tensor_copy(rrf, rrg)
            r_h1 = rrf[:, 1:2]
            r_h2 = rrf[:, 2:3]
            r_len = rrf[:, 12:13]

            # =========== assembly ===========
            # fact goals: substitute in remaining
            fg = small.tile([NSLOT, 24], F32)
            nc.vector.tensor_copy(fg, g_rem)
            msk = small.tile([NSLOT, 8], F32)
            mski = small.tile([NSLOT, 8], I32)
            gv_a = [g_qa0, g_qa1]
            gv_v = [g_va0, g_va1]
            for sub in range(2):
                bv = ft[:, 1 + sub:2 + sub]
                for a in range(2):
                    col = fg.rearrange("s (n c) -> s c n", c=3)[:, 1 + a]
                    nc.vector.scalar_tensor_tensor(
                        msk, col, gv_a[sub], gv_v[sub].to_broadcast([NSLOT, 8]),
                        op0=OP.is_equal, op1=OP.mult)
                    nc.vector.tensor_copy(mski, msk)
                    nc.vector.copy_predicated(col, mski, bv.to_broadcast([NSLOT, 8]))
            fgb = small.tile([NSLOT, 9], F32)
            nc.vector.memset(fgb, float(PAD))
            nc.vector.tensor_copy(fgb[:, 0:3], ft)

            # rule goals: bind body, mask by len, append rem[1:6]
            rb = small.tile([NSLOT, 9], F32)
            nc.vector.tensor_copy(rb, rrf[:, 3:12])
            msk3 = small.tile([NSLOT, 3], F32)
            msk3i = small.tile([NSLOT, 3], I32)
            cnd = small.tile([NSLOT, 1], F32)
            hvs = small.tile([NSLOT, 1], F32)
            for sub in range(2):
                hv = [r_h1, r_h2][sub]
                qvv = [g_qa0, g_qa1][sub]
                vaf = [g_va0, g_va1][sub]
                # cond scalar: is_var(hv) & ~is_var(qv) & qv != PAD
                nc.vector.tensor_scalar(hvs, hv, float(CONST_NO), None, op0=OP.is_gt)
                nc.vector.tensor_scalar(cnd, hv, float(PAD), None, op0=OP.not_equal)
                nc.vector.tensor_tensor(cnd, cnd, hvs, op=OP.mult)
                nc.vector.tensor_scalar(hvs, vaf, -1.0, 1.0, op0=OP.mult, op1=OP.add)
                nc.vector.tensor_tensor(cnd, cnd, hvs, op=OP.mult)
                nc.vector.tensor_scalar(hvs, qvv, float(PAD), None, op0=OP.not_equal)
                nc.vector.tensor_tensor(cnd, cnd, hvs, op=OP.mult)
                for a in range(2):
                    col = rb.rearrange("s (n c) -> s c n", c=3)[:, 1 + a]
                    nc.vector.scalar_tensor_tensor(
                        msk3, col, hv, cnd.to_broadcast([NSLOT, 3]),
                        op0=OP.is_equal, op1=OP.mult)
                    nc.vector.tensor_copy(msk3i, msk3)
                    nc.vector.copy_predicated(col, msk3i, qvv.to_broadcast([NSLOT, 3]))
            # len mask
            i9f = small.tile([NSLOT, 9], F32)
            nc.vector.memset(i9f[:, 0:3], 0.0)
            nc.vector.memset(i9f[:, 3:6], 1.0)
            nc.vector.memset(i9f[:, 6:9], 2.0)
            lm9 = small.tile([NSLOT, 9], I32)
            nc.vector.tensor_scalar(lm9, i9f, r_len, None, op0=OP.is_ge)
            padt9 = small.tile([NSLOT, 9], F32)
            nc.vector.memset(padt9, float(PAD))
            nc.vector.copy_predicated(rb, lm9, padt9)
            rg24 = small.tile([NSLOT, 24], F32)
            nc.vector.tensor_copy(rg24[:, 0:9], rb)
            nc.vector.tensor_copy(rg24[:, 9:24], g_rem[:, 3:18])
            rgb = small.tile([NSLOT, 9], F32)
            nc.vector.memset(rgb, float(PAD))
            nc.vector.tensor_copy(rgb[:, 0:3], srg[:, 0:3])

            # merge
            isfi = small.tile([NSLOT, 1], I32)
            nc.vector.tensor_copy(isfi, isfact)
            isri = small.tile([NSLOT, 1], I32)
            nc.vector.tensor_copy(isri, isrule)
            padt24 = small.tile([NSLOT, 24], F32)
            nc.vector.memset(padt24, float(PAD))
            gfin = small.tile([NSLOT, 24], F32)
            nc.vector.tensor_copy(gfin, padt24)
            nc.vector.copy_predicated(gfin, isri.to_broadcast([NSLOT, 24]), rg24)
            nc.vector.copy_predicated(gfin, isfi.to_broadcast([NSLOT, 24]), fg)
            bfin = small.tile([NSLOT, 9], F32)
            nc.vector.tensor_copy(bfin, padt9)
            nc.vector.copy_predicated(bfin, isri.to_broadcast([NSLOT, 9]), rgb)
            nc.vector.copy_predicated(bfin, isfi.to_broadcast([NSLOT, 9]), fgb)

            gi = small.tile([NSLOT, 24], I32)
            nc.vector.tensor_copy(gi, gfin)
            bi = small.tile([NSLOT, 9], I32)
            nc.vector.tensor_copy(bi, bfin)
            sco = small.tile([NSLOT, 1], F32)
            nc.vector.tensor_tensor(sco, g_sc, valid, op=OP.mult)
            vrow = small.tile([1, NSLOT], F32)
            pe_transpose(vrow, valid, ident, "vrow")
            virow = small.tile([1, NSLOT], I32)
            nc.vector.tensor_copy(virow, vrow)
            scrow2 = small.tile([1, NSLOT], F32)
            pe_transpose(scrow2, sco, ident, "scrow2")
            nc.sync.dma_start(goals_out[:, :], gi)
            nc.sync.dma_start(gbody_out[:, :], bi)
            nc.sync.dma_start(succ_out[:, :], virow)
            nc.sync.dma_start(scout_out[:, :], scrow2)

    return nc


_CACHED = {}


def _get_nc(Fp, nchunk):
    key = (Fp, nchunk)
    if key not in _CACHED:
        nc = build_kernel(Fp, nchunk)
        nc.finalize()
        _CACHED[key] = nc
    return _CACHED[key]


def make_in_maps(proof_goals, facts_idx, rules_heads_idx, rules_bodies_idx,
                 rule_lens, state_scores, Fp, nchunk=4):
    Fin = facts_idx.shape[0]
    facts_pad = np.full((Fp, 3), PADFACT, np.int32)
    facts_pad[:Fin] = facts_idx
    # device layout: per chunk, fact (j*128+p) at row (p*fpp + j) -> partition-minor
    parts = []
    off = 0
    for ch in chunk_sizes(Fp, nchunk):
        fpp = ch // 128
        parts.append(facts_pad[off:off + ch].reshape(fpp, 128, 3).transpose(1, 0, 2)
                     .reshape(ch, 3))
        off += ch
    facts_dev = np.ascontiguousarray(np.concatenate(parts, axis=0))
    factsg = np.zeros((Fp, 4), np.int32)
    factsg[:, :3] = facts_pad
    factsg = factsg.reshape(Fp // 16, 64)
    in_maps = []
    for b in range(B):
        in_maps.append({
            "pg": np.ascontiguousarray(proof_goals[b].reshape(S, G * 3)).astype(np.int32),
            "scores": np.ascontiguousarray(state_scores[b].reshape(1, S)).astype(np.float32),
            "facts": facts_dev,
            "factsg": factsg,
            "heads": np.ascontiguousarray(rules_heads_idx).astype(np.int32),
            "bodies": np.ascontiguousarray(rules_bodies_idx.reshape(R, 9)).astype(np.int32),
            "lens": np.ascontiguousarray(rule_lens.reshape(R, 1)).astype(np.int32),
        })
    return in_maps


def assemble(results):
    goals = np.stack([np.asarray(r["goals"]).reshape(S, G, 3) for r in results])
    gbody = np.stack([np.asarray(r["gbody"]).reshape(S, MB, 3) for r in results])
    succ = np.stack([np.asarray(r["succ"]).reshape(S) for r in results]).astype(bool)
    scores = np.stack([np.asarray(r["scout"]).reshape(S) for r in results]).astype(np.float32)
    return goals.astype(np.int32), gbody.astype(np.int32), succ, scores


def kernel(proof_goals, facts_idx, rules_heads_idx, rules_bodies_idx,
           rule_lens, state_scores):
    from concourse.bass_utils import run_bass_kernel_spmd
    Fp, nchunk = 100352, 2
    nc = _get_nc(Fp, nchunk)
    in_maps = make_in_maps(np.asarray(proof_goals), np.asarray(facts_idx),
                           np.asarray(rules_heads_idx), np.asarray(rules_bodies_idx),
                           np.asarray(rule_lens), np.asarray(state_scores), Fp, nchunk)
    res = run_bass_kernel_spmd(nc, in_maps, core_ids=list(range(8)))
    return assemble(res.results)
